# revision 3
# baseline (speedup 1.0000x reference)
"""Trainium2 Bass kernel for nn_A2EvULoss (EvU loss over [1M, 100] logits).

Data-parallel over 8 NeuronCores: each core streams its 125k-row shard once
from HBM, computing per-row (packed max+argmax, sum of exp), all-reduces the
uncertainty min/max, builds 21-threshold step masks and reduces the four
weighted sums with PSUM-accumulated matmuls, all-reduces those, and finishes
the 21-point trapezoid + log replicated on every core.

Per-row trick: one scalar_tensor_tensor packs the class index into the 7 low
mantissa bits of each logit ((x & ~0x7F) | (99 - c)); a single reduce_max then
yields both the row max (to 2^-17 relative) and the argmax.
"""

import numpy as np

P = 128
C = 100
N_CORES = 8
N_TOTAL = 1_000_000
NPC = N_TOTAL // N_CORES          # 125000 rows per core
MCOLS = NPC // P                  # 976 main stat columns (p-major layout)
REM = NPC - P * MCOLS             # 72 remainder rows
COLS = MCOLS + 1                  # 977 stat columns (col 976 = remainder)
T = 16                            # row-tiles per streaming chunk
NCHUNKS = MCOLS // T              # 61
NTH = 21
K = NTH + 1                       # 21 thresholds + 1 all-ones (totals) column
EPS = 1e-10

F32 = None  # filled lazily (mybir import kept inside functions)


def _build_nc():
    import concourse.bass as bass
    import concourse.bacc as bacc
    import concourse.tile as tile
    from concourse import mybir

    f32 = mybir.dt.float32
    i32 = mybir.dt.int32
    bf16 = mybir.dt.bfloat16
    Op = mybir.AluOpType

    nc = bacc.Bacc("TRN2", target_bir_lowering=False, debug=False,
                   num_devices=N_CORES)

    x_d = nc.dram_tensor("x", [NPC, C], f32, kind="ExternalInput")
    tgt_d = nc.dram_tensor("tgt", [P, COLS], i32, kind="ExternalInput")
    valid_d = nc.dram_tensor("valid", [P, COLS], f32, kind="ExternalInput")
    out_d = nc.dram_tensor("out", [1, 1], f32, kind="ExternalOutput")

    x_main = x_d.ap()[0:P * MCOLS, :].rearrange("(p c) f -> p (c f)", p=P)
    x_rem = x_d.ap()[P * MCOLS:NPC, :]                      # [72, 100]

    with tile.TileContext(nc) as tc:
        with (
            tc.tile_pool(name="stream", bufs=3) as stream,
            tc.tile_pool(name="persist", bufs=1) as persist,
            tc.tile_pool(name="psum", bufs=1, space="PSUM") as psump,
            tc.tile_pool(name="dram", bufs=1, space="DRAM") as dram,
        ):
            # ---- constants ----
            iota_big = persist.tile([P, T * C], i32)     # 99 - class, per tile
            nc.gpsimd.iota(iota_big[:], pattern=[[0, T], [-1, C]], base=99,
                           channel_multiplier=0)
            iota_k = persist.tile([P, K], i32)
            nc.gpsimd.iota(iota_k[:], pattern=[[1, K]], base=0,
                           channel_multiplier=0)
            iota_kf = persist.tile([P, K], f32)
            nc.vector.tensor_copy(iota_kf[:], iota_k[:])
            and_hi = persist.tile([P, 1], i32)
            nc.vector.memset(and_hi[:], -128)            # 0xFFFFFF80
            and_lo = persist.tile([P, 1], i32)
            nc.vector.memset(and_lo[:], 127)             # 0x0000007F

            tgt_sb = persist.tile([P, COLS], i32)
            nc.sync.dma_start(tgt_sb[:], tgt_d.ap())
            valid_sb = persist.tile([P, COLS], f32)
            nc.sync.dma_start(valid_sb[:], valid_d.ap())

            # ---- per-row stats ----
            pmax = persist.tile([P, COLS], f32)          # packed row max
            sumexp = persist.tile([P, COLS], f32)

            # ---- phase 1: stream x, compute packed max + sumexp ----
            for ch in range(NCHUNKS + 1):
                last = ch == NCHUNKS
                xt = stream.tile([P, T * C], f32, tag="xt")
                if last:
                    nc.vector.memset(xt[:, 0:C], 0.0)
                    nc.sync.dma_start(xt[0:REM, 0:C], x_rem)
                    xs = xt[:, 0:C].rearrange("p (t f) -> p t f", f=C)
                    io = iota_big[:, 0:C].rearrange("p (t f) -> p t f", f=C)
                    sl = slice(MCOLS, COLS)
                else:
                    nc.sync.dma_start(
                        xt[:], x_main[:, ch * T * C:(ch + 1) * T * C])
                    xs = xt[:].rearrange("p (t f) -> p t f", f=C)
                    io = iota_big[:].rearrange("p (t f) -> p t f", f=C)
                    sl = slice(ch * T, (ch + 1) * T)

                packed = stream.tile([P, T * C], i32, tag="packed")
                pk = packed[:, 0:xs.shape[1] * C].rearrange(
                    "p (t f) -> p t f", f=C)
                nc.vector.scalar_tensor_tensor(
                    pk, xs.bitcast(i32), and_hi[:], io,
                    op0=Op.bitwise_and, op1=Op.bitwise_or)
                nc.vector.reduce_max(
                    pmax[:, sl], pk.bitcast(f32), axis=mybir.AxisListType.X)

                y = stream.tile([P, T * C], f32, tag="y")
                ys = y[:, 0:xs.shape[1] * C]
                nc.scalar.activation(
                    ys, xt[:, 0:xs.shape[1] * C],
                    mybir.ActivationFunctionType.Exp)
                nc.vector.reduce_sum(
                    sumexp[:, sl], ys.rearrange("p (t f) -> p t f", f=C),
                    axis=mybir.AxisListType.X)

            # ---- phase 1b: per-row derived quantities ----
            # unc = C / (sumexp + C)
            sumalpha = persist.tile([P, COLS], f32)
            nc.vector.tensor_scalar(sumalpha[:], sumexp[:], float(C), None,
                                    Op.add)
            rcp = persist.tile([P, COLS], f32)
            nc.vector.reciprocal(rcp[:], sumalpha[:])
            unc = persist.tile([P, COLS], f32)
            nc.vector.tensor_scalar(unc[:], rcp[:], float(C), None, Op.mult)

            # local min/max of unc over valid rows
            bigmask = persist.tile([P, COLS], f32)       # 0 valid, 1e9 invalid
            nc.vector.tensor_scalar(bigmask[:], valid_sb[:], 1.0, -1e9,
                                    Op.subtract, Op.mult)
            umin_in = persist.tile([P, COLS], f32)
            nc.vector.tensor_add(umin_in[:], unc[:], bigmask[:])
            umax_in = persist.tile([P, COLS], f32)
            nc.vector.tensor_sub(umax_in[:], unc[:], bigmask[:])
            mm = persist.tile([P, 2], f32)               # (umax, -umin)
            nc.vector.reduce_max(mm[:, 0:1], umax_in[:],
                                 axis=mybir.AxisListType.X)
            umin_loc = persist.tile([P, 1], f32)
            nc.vector.tensor_reduce(umin_loc[:], umin_in[:],
                                    axis=mybir.AxisListType.X, op=Op.min)
            nc.vector.tensor_scalar(mm[:, 1:2], umin_loc[:], -1.0, None,
                                    Op.mult)
            # cross-partition: SBUF [128,2] -> SBUF [1,256], reduce over p
            mmx = persist.tile([1, 2 * P], f32)
            nc.sync.dma_start(mmx[:], mm[:])
            mmr = persist.tile([1, 2], f32)
            nc.vector.reduce_max(
                mmr[:],
                bass.AP(tensor=mmx.tensor, offset=mmx[:].offset,
                        ap=[list(mmx[:].ap[0]), [1, 2], [2, P]]),
                axis=mybir.AxisListType.X)

            # all-reduce (max) of (umax, -umin) across cores
            cc1_in = dram.tile([1, 2], f32)
            cc1_out = dram.tile([1, 2], f32)
            nc.sync.dma_start(cc1_in[:], mmr[:])
            nc.gpsimd.collective_compute(
                "AllReduce", Op.max,
                replica_groups=[list(range(N_CORES))],
                ins=[cc1_in[:].opt()], outs=[cc1_out[:].opt()])
            gmm = persist.tile([P, 2], f32)
            nc.gpsimd.dma_start(
                gmm[:],
                bass.AP(tensor=cc1_out.tensor, offset=cc1_out[:].offset,
                        ap=[[0, P], [1, 2]]))

            # weights (independent of the collective; overlaps with it)
            rclean = persist.tile([P, COLS], f32)
            nc.vector.tensor_scalar(rclean[:].bitcast(i32),
                                    pmax[:].bitcast(i32), and_hi[:], None,
                                    Op.bitwise_and)
            e = persist.tile([P, COLS], f32)             # max evidence
            nc.scalar.activation(e[:], rclean[:],
                                 mybir.ActivationFunctionType.Exp)
            t_ = persist.tile([P, COLS], f32)
            nc.scalar.activation(t_[:], unc[:],
                                 mybir.ActivationFunctionType.Tanh)
            omt = persist.tile([P, COLS], f32)           # 1 - t
            nc.vector.tensor_scalar(omt[:], t_[:], -1.0, 1.0, Op.mult, Op.add)

            idx_i = persist.tile([P, COLS], i32)
            nc.vector.tensor_scalar(idx_i[:], pmax[:].bitcast(i32),
                                    and_lo[:], None, Op.bitwise_and)
            corr_i = persist.tile([P, COLS], i32)
            nc.vector.tensor_tensor(corr_i[:], idx_i[:], tgt_sb[:],
                                    op=Op.is_equal)
            corr = persist.tile([P, COLS], f32)
            nc.vector.tensor_copy(corr[:], corr_i[:])
            nc.vector.tensor_mul(corr[:], corr[:], valid_sb[:])

            m1 = persist.tile([P, COLS], f32)            # correct: max_alpha
            nc.vector.scalar_tensor_tensor(m1[:], e[:], 1.0, corr[:],
                                           op0=Op.add, op1=Op.mult)
            m0 = persist.tile([P, COLS], f32)            # incorrect: 1-max_a
            cmv = persist.tile([P, COLS], f32)
            nc.vector.tensor_sub(cmv[:], corr[:], valid_sb[:])
            nc.vector.tensor_mul(m0[:], cmv[:], e[:])

            w4 = persist.tile([P, COLS, 4], bf16)
            nc.vector.tensor_mul(w4[:, :, 0], m1[:], omt[:])   # ac
            nc.vector.tensor_mul(w4[:, :, 1], m1[:], t_[:])    # au
            nc.vector.tensor_mul(w4[:, :, 2], m0[:], omt[:])   # ic
            nc.vector.tensor_mul(w4[:, :, 3], m0[:], t_[:])    # iu

            # bucket b = clamp((unc - umin) * 20 / (umax - umin), <= 20)
            rng = persist.tile([P, 1], f32)
            nc.vector.tensor_add(rng[:], gmm[:, 0:1], gmm[:, 1:2])
            rrng = persist.tile([P, 1], f32)
            nc.vector.reciprocal(rrng[:], rng[:])
            s1 = persist.tile([P, 1], f32)
            nc.vector.tensor_scalar(s1[:], rrng[:], float(NTH - 1), None,
                                    Op.mult)
            gmin = persist.tile([P, 1], f32)
            nc.vector.tensor_scalar(gmin[:], gmm[:, 1:2], -1.0, None, Op.mult)
            ub = persist.tile([P, COLS], f32)
            nc.vector.tensor_scalar(ub[:], unc[:], gmin[:], None, Op.subtract)
            b = persist.tile([P, COLS], f32)
            nc.vector.tensor_tensor(
                b[:], ub[:], s1[:].broadcast_to((P, COLS)), op=Op.mult)
            bc = persist.tile([P, COLS], f32)
            nc.vector.tensor_scalar(bc[:], b[:], float(NTH - 1), None, Op.min)

            # mask[p, c, k] = bc[p, c] <= k   (k = 0..20, col 21 always 1)
            mask = persist.tile([P, COLS, K], bf16)
            nc.vector.tensor_tensor(
                mask[:],
                bc[:].unsqueeze(2).broadcast_to((P, COLS, K)),
                iota_kf[:].unsqueeze(1).broadcast_to((P, COLS, K)),
                op=Op.is_le)
            w4b = w4  # already bf16

            # S[j, k] = sum_i w4[i, j] * mask[i, k], PSUM-accumulated
            S = psump.tile([4, K], f32)
            for c in range(COLS):
                nc.tensor.matmul(S[:], w4b[:, c, :], mask[:, c, :],
                                 start=(c == 0), stop=(c == COLS - 1))

            s_sb = persist.tile([4, K], f32)
            nc.vector.tensor_copy(s_sb[:], S[:])
            cc2_in = dram.tile([4, K], f32)
            cc2_out = dram.tile([4, K], f32)
            nc.sync.dma_start(cc2_in[:], s_sb[:])
            nc.gpsimd.collective_compute(
                "AllReduce", Op.add,
                replica_groups=[list(range(N_CORES))],
                ins=[cc2_in[:].opt()], outs=[cc2_out[:].opt()])

            # flatten [4, K] -> [1, 4K] on partition 0
            f = persist.tile([1, 4 * K], f32)
            nc.sync.dma_start(
                f[:], cc2_out[:].rearrange("a b -> (a b)").unsqueeze(0))

            ac = f[:, 0:NTH]
            aup = f[:, K:K + NTH]
            t_au = f[:, K + NTH:K + NTH + 1]
            ic = f[:, 2 * K:2 * K + NTH]
            iup = f[:, 3 * K:3 * K + NTH]
            t_iu = f[:, 3 * K + NTH:3 * K + NTH + 1]

            nneg_iu = persist.tile([1, NTH], f32)        # -n_iu
            nc.vector.tensor_scalar(nneg_iu[:], iup, t_iu, None, Op.subtract)
            num = persist.tile([1, NTH], f32)            # n_ac + n_iu
            nc.vector.tensor_sub(num[:], ac, nneg_iu[:])
            nneg_au = persist.tile([1, NTH], f32)        # -n_au
            nc.vector.tensor_scalar(nneg_au[:], aup, t_au, None, Op.subtract)
            den = persist.tile([1, NTH], f32)
            nc.vector.tensor_sub(den[:], num[:], nneg_au[:])
            nc.vector.tensor_add(den[:], den[:], ic)
            nc.vector.tensor_scalar(den[:], den[:], EPS, None, Op.add)
            rden = persist.tile([1, NTH], f32)
            nc.vector.reciprocal(rden[:], den[:])
            evu = persist.tile([1, NTH], f32)
            nc.vector.tensor_mul(evu[:], num[:], rden[:])

            ssum = persist.tile([1, 1], f32)
            nc.vector.reduce_sum(ssum[:], evu[:], axis=mybir.AxisListType.X)
            edge = persist.tile([1, 1], f32)
            nc.vector.tensor_add(edge[:], evu[:, 0:1], evu[:, NTH - 1:NTH])
            nc.vector.tensor_scalar(edge[:], edge[:], 0.5, None, Op.mult)
            auc = persist.tile([1, 1], f32)
            nc.vector.tensor_sub(auc[:], ssum[:], edge[:])
            nc.vector.tensor_scalar(auc[:], auc[:], 1.0 / (2 * (NTH - 1)) * 2,
                                    None, Op.mult)
            eps_t = persist.tile([1, 1], f32)
            nc.vector.memset(eps_t[:], EPS)
            nll = persist.tile([1, 1], f32)
            nc.scalar.activation(nll[:], auc[:],
                                 mybir.ActivationFunctionType.Ln,
                                 bias=eps_t[:])
            res = persist.tile([1, 1], f32)
            nc.vector.tensor_scalar(res[:], nll[:], -1.0, None, Op.mult)
            nc.sync.dma_start(out_d.ap(), res[:])

    nc.compile()
    return nc


_NC = None


def _get_nc():
    global _NC
    if _NC is None:
        _NC = _build_nc()
    return _NC


_VALID = None


def _valid_mask():
    global _VALID
    if _VALID is None:
        v = np.ones((P, COLS), np.float32)
        v[:, MCOLS] = 0.0
        v[:REM, MCOLS] = 1.0
        _VALID = v
    return _VALID


def _in_maps(output, target):
    output = np.ascontiguousarray(np.asarray(output, dtype=np.float32))
    target = np.asarray(target)
    maps = []
    v = _valid_mask()
    for i in range(N_CORES):
        xs = output[i * NPC:(i + 1) * NPC]
        ts = np.asarray(target[i * NPC:(i + 1) * NPC], dtype=np.int64)
        t99 = (99 - ts).astype(np.int32)
        tgt = np.empty((P, COLS), np.int32)
        tgt[:, :MCOLS] = t99[:P * MCOLS].reshape(P, MCOLS)
        tgt[:, MCOLS] = -1
        tgt[:REM, MCOLS] = t99[P * MCOLS:]
        maps.append({"x": xs, "tgt": tgt, "valid": v})
    return maps


def run(output, target, trace=False):
    from concourse.bass_utils import run_bass_kernel_spmd
    nc = _get_nc()
    res = run_bass_kernel_spmd(nc, _in_maps(output, target),
                               core_ids=list(range(N_CORES)), trace=trace)
    val = np.float32(res.results[0]["out"][0, 0])
    return val, res


def kernel(output, target, num_classes):
    assert int(num_classes) == C
    val, _ = run(output, target)
    return np.array(val, dtype=np.float32)


# revision 9
# speedup vs baseline: 1.0723x; 1.0723x over previous
"""Trainium2 Bass kernel for nn_A2EvULoss (EvU loss over [1M, 100] logits).

Data-parallel over 8 NeuronCores: each core streams its 125k-row shard once
from HBM, computing per-row (packed max+argmax, sum of exp), all-reduces the
uncertainty min/max, builds 21-threshold step masks and reduces the four
weighted sums with PSUM-accumulated matmuls, all-reduces those, and finishes
the 21-point trapezoid + log replicated on every core.

Per-row trick: one scalar_tensor_tensor packs the class index into the 7 low
mantissa bits of each logit ((x & ~0x7F) | (99 - c)); a single reduce_max then
yields both the row max (to 2^-17 relative) and the argmax.
"""

import numpy as np

P = 128
C = 100
N_CORES = 8
N_TOTAL = 1_000_000
NPC = N_TOTAL // N_CORES          # 125000 rows per core
MCOLS = NPC // P                  # 976 main stat columns (p-major layout)
REM = NPC - P * MCOLS             # 72 remainder rows
COLS = MCOLS + 1                  # 977 stat columns (col 976 = remainder)
T = 16                            # row-tiles per streaming chunk
NB_DVE = 6                        # tiles per chunk whose sumexp uses DVE reduce
NCHUNKS = MCOLS // T              # 61
NTH = 21
K = NTH + 1                       # 21 thresholds + 1 all-ones (totals) column
EPS = 1e-10

F32 = None  # filled lazily (mybir import kept inside functions)


def _build_nc():
    import concourse.bass as bass
    import concourse.bacc as bacc
    import concourse.tile as tile
    from concourse import mybir

    f32 = mybir.dt.float32
    i32 = mybir.dt.int32
    bf16 = mybir.dt.bfloat16
    Op = mybir.AluOpType

    nc = bacc.Bacc("TRN2", target_bir_lowering=False, debug=False,
                   num_devices=N_CORES)

    x_d = nc.dram_tensor("x", [NPC, C], f32, kind="ExternalInput")
    tgt_d = nc.dram_tensor("tgt", [P, COLS], i32, kind="ExternalInput")
    valid_d = nc.dram_tensor("valid", [P, COLS], f32, kind="ExternalInput")
    out_d = nc.dram_tensor("out", [1, 1], f32, kind="ExternalOutput")

    x_main = x_d.ap()[0:P * MCOLS, :].rearrange("(p c) f -> p (c f)", p=P)
    x_rem = x_d.ap()[P * MCOLS:NPC, :]                      # [72, 100]

    with tile.TileContext(nc) as tc:
        with (
            tc.tile_pool(name="stream", bufs=3) as stream,
            tc.tile_pool(name="persist", bufs=1) as persist,
            tc.tile_pool(name="psum", bufs=1, space="PSUM") as psump,
            tc.tile_pool(name="dram", bufs=1, space="DRAM") as dram,
        ):
            # ---- constants ----
            iota_big = persist.tile([P, T * C], i32)     # 99 - class, per tile
            nc.gpsimd.iota(iota_big[:], pattern=[[0, T], [-1, C]], base=99,
                           channel_multiplier=0)
            iota_k = persist.tile([P, K], i32)
            nc.gpsimd.iota(iota_k[:], pattern=[[1, K]], base=0,
                           channel_multiplier=0)
            iota_kf = persist.tile([P, K], f32)
            nc.vector.tensor_copy(iota_kf[:], iota_k[:])
            and_hi = persist.tile([P, 1], i32)
            nc.vector.memset(and_hi[:], -128)            # 0xFFFFFF80
            and_lo = persist.tile([P, 1], i32)
            nc.vector.memset(and_lo[:], 127)             # 0x0000007F

            tgt_sb = persist.tile([P, COLS], i32)
            nc.sync.dma_start(tgt_sb[:], tgt_d.ap())
            valid_sb = persist.tile([P, COLS], f32)
            nc.sync.dma_start(valid_sb[:], valid_d.ap())

            # ---- per-row stats ----
            pmax = persist.tile([P, COLS], f32)          # packed row max
            sumexp = persist.tile([P, COLS], f32)

            # ---- phase 1: stream x, compute packed max + sumexp ----
            for ch in range(NCHUNKS + 1):
                last = ch == NCHUNKS
                xt = stream.tile([P, T * C], f32, tag="xt")
                if last:
                    nc.vector.memset(xt[:, 0:C], 0.0)
                    nc.sync.dma_start(xt[0:REM, 0:C], x_rem)
                    xs = xt[:, 0:C].rearrange("p (t f) -> p t f", f=C)
                    io = iota_big[:, 0:C].rearrange("p (t f) -> p t f", f=C)
                    sl = slice(MCOLS, COLS)
                else:
                    nc.sync.dma_start(
                        xt[:], x_main[:, ch * T * C:(ch + 1) * T * C])
                    xs = xt[:].rearrange("p (t f) -> p t f", f=C)
                    io = iota_big[:].rearrange("p (t f) -> p t f", f=C)
                    sl = slice(ch * T, (ch + 1) * T)

                packed = stream.tile([P, T * C], i32, tag="packed")
                pk = packed[:, 0:xs.shape[1] * C].rearrange(
                    "p (t f) -> p t f", f=C)
                nc.vector.scalar_tensor_tensor(
                    pk, xs.bitcast(i32), and_hi[:], io,
                    op0=Op.bitwise_and, op1=Op.bitwise_or)
                nc.vector.reduce_max(
                    pmax[:, sl], pk.bitcast(f32), axis=mybir.AxisListType.X)

                # sumexp: split between DVE (batched exp + reduce) and ACT
                # (per-tile exp with accumulator) to balance the two engines
                nt = xs.shape[1]
                nb = min(NB_DVE, nt)        # tiles summed on DVE
                y = stream.tile([P, T * C], f32, tag="y")
                nc.scalar.activation(
                    y[:, 0:nb * C], xt[:, 0:nb * C],
                    mybir.ActivationFunctionType.Exp)
                nc.vector.reduce_sum(
                    sumexp[:, sl.start:sl.start + nb],
                    y[:, 0:nb * C].rearrange("p (t f) -> p t f", f=C),
                    axis=mybir.AxisListType.X)
                for t in range(nb, nt):
                    nc.scalar.activation(
                        y[:, t * C:(t + 1) * C], xt[:, t * C:(t + 1) * C],
                        mybir.ActivationFunctionType.Exp,
                        accum_out=sumexp[:, sl.start + t:sl.start + t + 1])

            # ---- phase 1b: per-row derived quantities ----
            c100 = persist.tile([P, 1], f32)
            nc.vector.memset(c100[:], float(C))
            c1 = persist.tile([P, 1], f32)
            nc.vector.memset(c1[:], 1.0)

            # unc = C / (sumexp + C)
            sumalpha = persist.tile([P, COLS], f32)
            nc.scalar.activation(sumalpha[:], sumexp[:],
                                 mybir.ActivationFunctionType.Identity,
                                 bias=c100[:])
            rcp = persist.tile([P, COLS], f32)
            nc.vector.reciprocal(rcp[:], sumalpha[:])
            unc = persist.tile([P, COLS], f32)
            nc.scalar.mul(unc[:], rcp[:], float(C))

            # local min/max of unc over valid rows; issue collective early
            bigmask = persist.tile([P, COLS], f32)       # 0 valid, 1e9 invalid
            nc.scalar.activation(bigmask[:], valid_sb[:],
                                 mybir.ActivationFunctionType.Copy,
                                 bias=1e9, scale=-1e9)
            umin_in = persist.tile([P, COLS], f32)
            nc.vector.tensor_add(umin_in[:], unc[:], bigmask[:])
            umax_in = persist.tile([P, COLS], f32)
            nc.vector.tensor_sub(umax_in[:], unc[:], bigmask[:])
            mm = persist.tile([P, 2], f32)               # (umax, -umin)
            nc.vector.reduce_max(mm[:, 0:1], umax_in[:],
                                 axis=mybir.AxisListType.X)
            umin_loc = persist.tile([P, 1], f32)
            nc.vector.tensor_reduce(umin_loc[:], umin_in[:],
                                    axis=mybir.AxisListType.X, op=Op.min)
            nc.vector.tensor_scalar(mm[:, 1:2], umin_loc[:], -1.0, None,
                                    Op.mult)
            # cross-partition: SBUF [128,2] -> SBUF [1,256], reduce over p
            mmx = persist.tile([1, 2 * P], f32)
            nc.sync.dma_start(mmx[:], mm[:])
            mmr = persist.tile([1, 2], f32)
            nc.vector.reduce_max(
                mmr[:],
                bass.AP(tensor=mmx.tensor, offset=mmx[:].offset,
                        ap=[list(mmx[:].ap[0]), [1, 2], [2, P]]),
                axis=mybir.AxisListType.X)

            # all-reduce (max) of (umax, -umin) across cores
            cc1_in = dram.tile([1, 2], f32)
            cc1_out = dram.tile([1, 2], f32)
            nc.sync.dma_start(cc1_in[:], mmr[:])
            nc.gpsimd.collective_compute(
                "AllReduce", Op.max,
                replica_groups=[list(range(N_CORES))],
                ins=[cc1_in[:].opt()], outs=[cc1_out[:].opt()])
            gmm = persist.tile([P, 2], f32)
            nc.gpsimd.dma_start(
                gmm[:],
                bass.AP(tensor=cc1_out.tensor, offset=cc1_out[:].offset,
                        ap=[[0, P], [1, 2]]))

            # weights (independent of the collective; overlaps with it)
            rclean = persist.tile([P, COLS], f32)
            nc.vector.tensor_scalar(rclean[:].bitcast(i32),
                                    pmax[:].bitcast(i32), and_hi[:], None,
                                    Op.bitwise_and)
            e = persist.tile([P, COLS], f32)             # max evidence
            nc.scalar.activation(e[:], rclean[:],
                                 mybir.ActivationFunctionType.Exp)
            t_ = persist.tile([P, COLS], f32)
            nc.scalar.activation(t_[:], unc[:],
                                 mybir.ActivationFunctionType.Tanh)
            omt = persist.tile([P, COLS], f32)           # 1 - t
            nc.scalar.activation(omt[:], t_[:],
                                 mybir.ActivationFunctionType.Identity,
                                 bias=c1[:], scale=-1.0)

            idx_i = persist.tile([P, COLS], i32)
            nc.vector.tensor_scalar(idx_i[:], pmax[:].bitcast(i32),
                                    and_lo[:], None, Op.bitwise_and)
            corr = persist.tile([P, COLS], f32)          # pad tgt=-1 -> 0
            nc.vector.tensor_tensor(corr[:], idx_i[:], tgt_sb[:],
                                    op=Op.is_equal)

            m1 = persist.tile([P, COLS], f32)            # correct: max_alpha
            nc.vector.scalar_tensor_tensor(m1[:], e[:], 1.0, corr[:],
                                           op0=Op.add, op1=Op.mult)
            m0 = persist.tile([P, COLS], f32)            # incorrect: 1-max_a
            cmv = persist.tile([P, COLS], f32)
            nc.vector.tensor_sub(cmv[:], corr[:], valid_sb[:])
            nc.vector.tensor_mul(m0[:], cmv[:], e[:])

            w4 = persist.tile([P, COLS, 4], bf16)
            nc.vector.tensor_mul(w4[:, :, 0], m1[:], omt[:])   # ac
            nc.vector.tensor_mul(w4[:, :, 1], m1[:], t_[:])    # au
            nc.vector.tensor_mul(w4[:, :, 2], m0[:], omt[:])   # ic
            nc.vector.tensor_mul(w4[:, :, 3], m0[:], t_[:])    # iu

            # bucket b = clamp((unc - umin) * 20 / (umax - umin), <= 20)
            rng = persist.tile([P, 1], f32)
            nc.vector.tensor_add(rng[:], gmm[:, 0:1], gmm[:, 1:2])
            rrng = persist.tile([P, 1], f32)
            nc.vector.reciprocal(rrng[:], rng[:])
            s1 = persist.tile([P, 1], f32)
            nc.vector.tensor_scalar(s1[:], rrng[:], float(NTH - 1), None,
                                    Op.mult)
            gmin = persist.tile([P, 1], f32)
            nc.vector.tensor_scalar(gmin[:], gmm[:, 1:2], -1.0, None, Op.mult)
            ub = persist.tile([P, COLS], f32)
            nc.vector.tensor_scalar(ub[:], unc[:], gmin[:], None, Op.subtract)
            b = persist.tile([P, COLS], f32)
            nc.vector.tensor_tensor(
                b[:], ub[:], s1[:].broadcast_to((P, COLS)), op=Op.mult)
            bc = persist.tile([P, COLS], f32)
            nc.vector.tensor_scalar(bc[:], b[:], float(NTH - 1), None, Op.min)

            # mask[p, c, k] = bc[p, c] <= k, built in chunks so the PE can
            # start accumulating while later chunks are still being built
            S = psump.tile([4, K], f32)
            MCH = 8
            edges = [round(i * COLS / MCH) for i in range(MCH + 1)]
            with tc.tile_pool(name="maskp", bufs=2) as maskp:
                for mi in range(MCH):
                    c0, c1e = edges[mi], edges[mi + 1]
                    w = c1e - c0
                    mask = maskp.tile([P, -(-COLS // MCH), K], bf16,
                                      tag="mask")
                    nc.vector.tensor_tensor(
                        mask[:, 0:w, :],
                        bc[:, c0:c1e].unsqueeze(2).broadcast_to((P, w, K)),
                        iota_kf[:].unsqueeze(1).broadcast_to((P, w, K)),
                        op=Op.is_le)
                    for c in range(c0, c1e):
                        nc.tensor.matmul(S[:], w4[:, c, :],
                                         mask[:, c - c0, :],
                                         start=(c == 0), stop=(c == COLS - 1))

            s_sb = persist.tile([4, K], f32)
            nc.vector.tensor_copy(s_sb[:], S[:])
            cc2_in = dram.tile([4, K], f32)
            cc2_out = dram.tile([4, K], f32)
            nc.sync.dma_start(cc2_in[:], s_sb[:])
            nc.gpsimd.collective_compute(
                "AllReduce", Op.add,
                replica_groups=[list(range(N_CORES))],
                ins=[cc2_in[:].opt()], outs=[cc2_out[:].opt()])

            # flatten [4, K] -> [1, 4K] on partition 0
            f = persist.tile([1, 4 * K], f32)
            nc.sync.dma_start(
                f[:], cc2_out[:].rearrange("a b -> (a b)").unsqueeze(0))

            ac = f[:, 0:NTH]
            aup = f[:, K:K + NTH]
            t_au = f[:, K + NTH:K + NTH + 1]
            ic = f[:, 2 * K:2 * K + NTH]
            iup = f[:, 3 * K:3 * K + NTH]
            t_iu = f[:, 3 * K + NTH:3 * K + NTH + 1]

            nneg_iu = persist.tile([1, NTH], f32)        # -n_iu
            nc.vector.tensor_scalar(nneg_iu[:], iup, t_iu, None, Op.subtract)
            num = persist.tile([1, NTH], f32)            # n_ac + n_iu
            nc.vector.tensor_sub(num[:], ac, nneg_iu[:])
            nneg_au = persist.tile([1, NTH], f32)        # -n_au
            nc.vector.tensor_scalar(nneg_au[:], aup, t_au, None, Op.subtract)
            den = persist.tile([1, NTH], f32)
            nc.vector.tensor_sub(den[:], num[:], nneg_au[:])
            nc.vector.tensor_add(den[:], den[:], ic)
            nc.vector.tensor_scalar(den[:], den[:], EPS, None, Op.add)
            rden = persist.tile([1, NTH], f32)
            nc.vector.reciprocal(rden[:], den[:])
            evu = persist.tile([1, NTH], f32)
            nc.vector.tensor_mul(evu[:], num[:], rden[:])

            ssum = persist.tile([1, 1], f32)
            nc.vector.reduce_sum(ssum[:], evu[:], axis=mybir.AxisListType.X)
            edge = persist.tile([1, 1], f32)
            nc.vector.tensor_add(edge[:], evu[:, 0:1], evu[:, NTH - 1:NTH])
            nc.vector.tensor_scalar(edge[:], edge[:], 0.5, None, Op.mult)
            auc = persist.tile([1, 1], f32)
            nc.vector.tensor_sub(auc[:], ssum[:], edge[:])
            nc.vector.tensor_scalar(auc[:], auc[:], 1.0 / (2 * (NTH - 1)) * 2,
                                    None, Op.mult)
            eps_t = persist.tile([1, 1], f32)
            nc.vector.memset(eps_t[:], EPS)
            nll = persist.tile([1, 1], f32)
            nc.scalar.activation(nll[:], auc[:],
                                 mybir.ActivationFunctionType.Ln,
                                 bias=eps_t[:])
            res = persist.tile([1, 1], f32)
            nc.vector.tensor_scalar(res[:], nll[:], -1.0, None, Op.mult)
            nc.sync.dma_start(out_d.ap(), res[:])

    nc.compile()
    return nc


_NC = None


def _get_nc():
    global _NC
    if _NC is None:
        _NC = _build_nc()
    return _NC


_VALID = None


def _valid_mask():
    global _VALID
    if _VALID is None:
        v = np.ones((P, COLS), np.float32)
        v[:, MCOLS] = 0.0
        v[:REM, MCOLS] = 1.0
        _VALID = v
    return _VALID


def _in_maps(output, target):
    output = np.ascontiguousarray(np.asarray(output, dtype=np.float32))
    target = np.asarray(target)
    maps = []
    v = _valid_mask()
    for i in range(N_CORES):
        xs = output[i * NPC:(i + 1) * NPC]
        ts = np.asarray(target[i * NPC:(i + 1) * NPC], dtype=np.int64)
        t99 = (99 - ts).astype(np.int32)
        tgt = np.empty((P, COLS), np.int32)
        tgt[:, :MCOLS] = t99[:P * MCOLS].reshape(P, MCOLS)
        tgt[:, MCOLS] = -1
        tgt[:REM, MCOLS] = t99[P * MCOLS:]
        maps.append({"x": xs, "tgt": tgt, "valid": v})
    return maps


def run(output, target, trace=False):
    from concourse.bass_utils import run_bass_kernel_spmd
    nc = _get_nc()
    res = run_bass_kernel_spmd(nc, _in_maps(output, target),
                               core_ids=list(range(N_CORES)), trace=trace)
    val = np.float32(res.results[0]["out"][0, 0])
    return val, res


def kernel(output, target, num_classes):
    assert int(num_classes) == C
    val, _ = run(output, target)
    return np.array(val, dtype=np.float32)


# revision 11
# speedup vs baseline: 1.1153x; 1.0401x over previous
"""Trainium2 Bass kernel for nn_A2EvULoss (EvU loss over [1M, 100] logits).

Data-parallel over 8 NeuronCores: each core streams its 125k-row shard once
from HBM, computing per-row (packed max+argmax, sum of exp), all-reduces the
uncertainty min/max, builds 21-threshold step masks and reduces the four
weighted sums with PSUM-accumulated matmuls, all-reduces those, and finishes
the 21-point trapezoid + log replicated on every core.

Per-row trick: one scalar_tensor_tensor packs the class index into the 7 low
mantissa bits of each logit ((x & ~0x7F) | (99 - c)); a single reduce_max then
yields both the row max (to 2^-17 relative) and the argmax.
"""

import numpy as np

P = 128
C = 100
N_CORES = 8
N_TOTAL = 1_000_000
NPC = N_TOTAL // N_CORES          # 125000 rows per core
MCOLS = NPC // P                  # 976 main stat columns (p-major layout)
REM = NPC - P * MCOLS             # 72 remainder rows
COLS = MCOLS + 1                  # 977 stat columns (col 976 = remainder)
T = 16                            # row-tiles per streaming chunk
NB_DVE = 8                        # tiles per chunk whose sumexp uses DVE reduce
NCHUNKS = MCOLS // T              # 61
NTH = 21
K = NTH + 1                       # 21 thresholds + 1 all-ones (totals) column
EPS = 1e-10

F32 = None  # filled lazily (mybir import kept inside functions)


def _build_nc():
    import concourse.bass as bass
    import concourse.bacc as bacc
    import concourse.tile as tile
    from concourse import mybir

    f32 = mybir.dt.float32
    i32 = mybir.dt.int32
    bf16 = mybir.dt.bfloat16
    Op = mybir.AluOpType

    nc = bacc.Bacc("TRN2", target_bir_lowering=False, debug=False,
                   num_devices=N_CORES)

    x_d = nc.dram_tensor("x", [NPC, C], f32, kind="ExternalInput")
    tgt_d = nc.dram_tensor("tgt", [P, COLS], i32, kind="ExternalInput")
    valid_d = nc.dram_tensor("valid", [P, COLS], f32, kind="ExternalInput")
    out_d = nc.dram_tensor("out", [1, 1], f32, kind="ExternalOutput")

    x_main = x_d.ap()[0:P * MCOLS, :].rearrange("(p c) f -> p (c f)", p=P)
    x_rem = x_d.ap()[P * MCOLS:NPC, :]                      # [72, 100]

    with tile.TileContext(nc) as tc:
        with (
            tc.tile_pool(name="stream", bufs=3) as stream,
            tc.tile_pool(name="persist", bufs=1) as persist,
            tc.tile_pool(name="psum", bufs=1, space="PSUM") as psump,
            tc.tile_pool(name="dram", bufs=1, space="DRAM") as dram,
        ):
            # ---- constants ----
            iota_big = persist.tile([P, T * C], i32)     # 99 - class, per tile
            nc.gpsimd.iota(iota_big[:], pattern=[[0, T], [-1, C]], base=99,
                           channel_multiplier=0)
            iota_k = persist.tile([P, K], i32)
            nc.gpsimd.iota(iota_k[:], pattern=[[1, K]], base=0,
                           channel_multiplier=0)
            iota_kf = persist.tile([P, K], f32)
            nc.vector.tensor_copy(iota_kf[:], iota_k[:])
            and_hi = persist.tile([P, 1], i32)
            nc.vector.memset(and_hi[:], -128)            # 0xFFFFFF80
            and_lo = persist.tile([P, 1], i32)
            nc.vector.memset(and_lo[:], 127)             # 0x0000007F

            tgt_sb = persist.tile([P, COLS], i32)
            nc.sync.dma_start(tgt_sb[:], tgt_d.ap())
            valid_sb = persist.tile([P, COLS], f32)
            nc.sync.dma_start(valid_sb[:], valid_d.ap())

            # ---- per-row stats ----
            pmax = persist.tile([P, COLS], f32)          # packed row max
            sumexp = persist.tile([P, COLS], f32)
            run_lo = persist.tile([P, 1], f32)           # min sumexp (valid)
            nc.vector.memset(run_lo[:], 1e30)
            run_hi = persist.tile([P, 1], f32)           # max sumexp (valid)
            nc.vector.memset(run_hi[:], -1e30)

            # ---- phase 1: stream x, compute packed max + sumexp ----
            for ch in range(NCHUNKS + 1):
                last = ch == NCHUNKS
                xt = stream.tile([P, T * C], f32, tag="xt")
                if last:
                    nc.vector.memset(xt[:, 0:C], 0.0)
                    nc.sync.dma_start(xt[0:REM, 0:C], x_rem)
                    xs = xt[:, 0:C].rearrange("p (t f) -> p t f", f=C)
                    io = iota_big[:, 0:C].rearrange("p (t f) -> p t f", f=C)
                    sl = slice(MCOLS, COLS)
                else:
                    nc.sync.dma_start(
                        xt[:], x_main[:, ch * T * C:(ch + 1) * T * C])
                    xs = xt[:].rearrange("p (t f) -> p t f", f=C)
                    io = iota_big[:].rearrange("p (t f) -> p t f", f=C)
                    sl = slice(ch * T, (ch + 1) * T)

                nt = xs.shape[1]
                packed = stream.tile([P, T * C], i32, tag="packed")
                nc.vector.scalar_tensor_tensor(
                    packed[:, 0:nt * C], xt[:, 0:nt * C].bitcast(i32),
                    and_hi[:], iota_big[:, 0:nt * C],
                    op0=Op.bitwise_and, op1=Op.bitwise_or)
                nc.vector.reduce_max(
                    pmax[:, sl],
                    packed[:, 0:nt * C].bitcast(f32).rearrange(
                        "p (t f) -> p t f", f=C),
                    axis=mybir.AxisListType.X)

                # sumexp: split between DVE (batched exp + reduce) and ACT
                # (per-tile exp with accumulator) to balance the two engines
                nb = min(NB_DVE, nt)        # tiles summed on DVE
                y = stream.tile([P, T * C], f32, tag="y")
                nc.scalar.activation(
                    y[:, 0:nb * C], xt[:, 0:nb * C],
                    mybir.ActivationFunctionType.Exp)
                nc.vector.reduce_sum(
                    sumexp[:, sl.start:sl.start + nb],
                    y[:, 0:nb * C].rearrange("p (t f) -> p t f", f=C),
                    axis=mybir.AxisListType.X)
                for t in range(nb, nt):
                    nc.scalar.activation(
                        y[:, t * C:(t + 1) * C], xt[:, t * C:(t + 1) * C],
                        mybir.ActivationFunctionType.Exp,
                        accum_out=sumexp[:, sl.start + t:sl.start + t + 1])

                # running min/max of sumexp (unc is monotone in sumalpha)
                if last:
                    padb = stream.tile([P, 1], f32, tag="padb")
                    nc.scalar.activation(padb[:], valid_sb[:, MCOLS:COLS],
                                         mybir.ActivationFunctionType.Copy,
                                         bias=1e9, scale=-1e9)
                    chl_t = stream.tile([P, 1], f32, tag="chl")
                    nc.vector.tensor_add(chl_t[:], sumexp[:, MCOLS:COLS],
                                         padb[:])
                    chh_t = stream.tile([P, 1], f32, tag="chh")
                    nc.vector.tensor_sub(chh_t[:], sumexp[:, MCOLS:COLS],
                                         padb[:])
                    chl, chh = chl_t[:], chh_t[:]
                else:
                    chl = stream.tile([P, 1], f32, tag="chl")
                    nc.vector.tensor_reduce(chl[:], sumexp[:, sl],
                                            axis=mybir.AxisListType.X,
                                            op=Op.min)
                    chh = stream.tile([P, 1], f32, tag="chh")
                    nc.vector.reduce_max(chh[:], sumexp[:, sl],
                                         axis=mybir.AxisListType.X)
                    chl, chh = chl[:], chh[:]
                nc.vector.tensor_tensor(run_lo[:], run_lo[:], chl, op=Op.min)
                nc.vector.tensor_tensor(run_hi[:], run_hi[:], chh, op=Op.max)

            # ---- phase 1b: per-row derived quantities ----
            c100 = persist.tile([P, 1], f32)
            nc.vector.memset(c100[:], float(C))
            c1 = persist.tile([P, 1], f32)
            nc.vector.memset(c1[:], 1.0)

            # unc = C / (sumexp + C)
            sumalpha = persist.tile([P, COLS], f32)
            nc.scalar.activation(sumalpha[:], sumexp[:],
                                 mybir.ActivationFunctionType.Identity,
                                 bias=c100[:])
            rcp = persist.tile([P, COLS], f32)
            nc.vector.reciprocal(rcp[:], sumalpha[:])
            unc = persist.tile([P, COLS], f32)
            nc.scalar.mul(unc[:], rcp[:], float(C))

            # umax = C/(min sumalpha) etc: send (1/min_sa, -1/max_sa),
            # recover after the all-reduce.  max over cores of 1/min_sa
            # = 1/(global min sa).
            mm = persist.tile([P, 2], f32)
            nc.vector.tensor_scalar(mm[:, 0:1], run_lo[:], float(C), None,
                                    Op.add)
            nc.vector.tensor_scalar(mm[:, 1:2], run_hi[:], float(C), None,
                                    Op.add)
            mmi = persist.tile([P, 2], f32)
            nc.vector.reciprocal(mmi[:], mm[:])
            nc.vector.tensor_scalar(mmi[:, 1:2], mmi[:, 1:2], -1.0, None,
                                    Op.mult)
            # cross-partition: SBUF [128,2] -> SBUF [1,256], reduce over p
            mmx = persist.tile([1, 2 * P], f32)
            nc.sync.dma_start(mmx[:], mmi[:])
            mmr = persist.tile([1, 2], f32)
            nc.vector.reduce_max(
                mmr[:],
                bass.AP(tensor=mmx.tensor, offset=mmx[:].offset,
                        ap=[list(mmx[:].ap[0]), [1, 2], [2, P]]),
                axis=mybir.AxisListType.X)

            # all-reduce (max) of (umax, -umin) across cores
            cc1_in = dram.tile([1, 2], f32)
            cc1_out = dram.tile([1, 2], f32)
            nc.sync.dma_start(cc1_in[:], mmr[:])
            nc.gpsimd.collective_compute(
                "AllReduce", Op.max,
                replica_groups=[list(range(N_CORES))],
                ins=[cc1_in[:].opt()], outs=[cc1_out[:].opt()])
            gmm = persist.tile([P, 2], f32)
            nc.gpsimd.dma_start(
                gmm[:],
                bass.AP(tensor=cc1_out.tensor, offset=cc1_out[:].offset,
                        ap=[[0, P], [1, 2]]))

            # weights (independent of the collective; overlaps with it)
            rclean = persist.tile([P, COLS], f32)
            nc.vector.tensor_scalar(rclean[:].bitcast(i32),
                                    pmax[:].bitcast(i32), and_hi[:], None,
                                    Op.bitwise_and)
            e = persist.tile([P, COLS], f32)             # max evidence
            nc.scalar.activation(e[:], rclean[:],
                                 mybir.ActivationFunctionType.Exp)
            t_ = persist.tile([P, COLS], f32)
            nc.scalar.activation(t_[:], unc[:],
                                 mybir.ActivationFunctionType.Tanh)
            omt = persist.tile([P, COLS], f32)           # 1 - t
            nc.scalar.activation(omt[:], t_[:],
                                 mybir.ActivationFunctionType.Identity,
                                 bias=c1[:], scale=-1.0)

            idx_i = persist.tile([P, COLS], i32)
            nc.vector.tensor_scalar(idx_i[:], pmax[:].bitcast(i32),
                                    and_lo[:], None, Op.bitwise_and)
            corr = persist.tile([P, COLS], f32)          # pad tgt=-1 -> 0
            nc.vector.tensor_tensor(corr[:], idx_i[:], tgt_sb[:],
                                    op=Op.is_equal)

            m1 = persist.tile([P, COLS], f32)            # correct: max_alpha
            nc.vector.scalar_tensor_tensor(m1[:], e[:], 1.0, corr[:],
                                           op0=Op.add, op1=Op.mult)
            m0 = persist.tile([P, COLS], f32)            # incorrect: 1-max_a
            cmv = persist.tile([P, COLS], f32)
            nc.vector.tensor_sub(cmv[:], corr[:], valid_sb[:])
            nc.vector.tensor_mul(m0[:], cmv[:], e[:])

            w4 = persist.tile([P, COLS, 4], bf16)
            nc.vector.tensor_mul(w4[:, :, 0], m1[:], omt[:])   # ac
            nc.vector.tensor_mul(w4[:, :, 1], m1[:], t_[:])    # au
            nc.vector.tensor_mul(w4[:, :, 2], m0[:], omt[:])   # ic
            nc.vector.tensor_mul(w4[:, :, 3], m0[:], t_[:])    # iu

            # gmm holds (1/min_sa, -1/max_sa) -> (umax, -umin) after *C
            nc.vector.tensor_scalar(gmm[:], gmm[:], float(C), None, Op.mult)
            # bucket b = clamp((unc - umin) * 20 / (umax - umin), <= 20)
            rng = persist.tile([P, 1], f32)
            nc.vector.tensor_add(rng[:], gmm[:, 0:1], gmm[:, 1:2])
            rrng = persist.tile([P, 1], f32)
            nc.vector.reciprocal(rrng[:], rng[:])
            s1 = persist.tile([P, 1], f32)
            nc.vector.tensor_scalar(s1[:], rrng[:], float(NTH - 1), None,
                                    Op.mult)
            gmin = persist.tile([P, 1], f32)
            nc.vector.tensor_scalar(gmin[:], gmm[:, 1:2], -1.0, None, Op.mult)
            ub = persist.tile([P, COLS], f32)
            nc.vector.tensor_scalar(ub[:], unc[:], gmin[:], None, Op.subtract)
            b = persist.tile([P, COLS], f32)
            nc.vector.tensor_tensor(
                b[:], ub[:], s1[:].broadcast_to((P, COLS)), op=Op.mult)
            bc = persist.tile([P, COLS], f32)
            nc.vector.tensor_scalar(bc[:], b[:], float(NTH - 1), None, Op.min)

            # mask[p, c, k] = bc[p, c] <= k, built in chunks so the PE can
            # start accumulating while later chunks are still being built
            S = psump.tile([4, K], f32)
            MCH = 16
            edges = [round(i * COLS / MCH) for i in range(MCH + 1)]
            with tc.tile_pool(name="maskp", bufs=2) as maskp:
                for mi in range(MCH):
                    c0, c1e = edges[mi], edges[mi + 1]
                    w = c1e - c0
                    mask = maskp.tile([P, -(-COLS // MCH), K], bf16,
                                      tag="mask")
                    nc.vector.tensor_tensor(
                        mask[:, 0:w, :],
                        bc[:, c0:c1e].unsqueeze(2).broadcast_to((P, w, K)),
                        iota_kf[:].unsqueeze(1).broadcast_to((P, w, K)),
                        op=Op.is_le)
                    for c in range(c0, c1e):
                        nc.tensor.matmul(S[:], w4[:, c, :],
                                         mask[:, c - c0, :],
                                         start=(c == 0), stop=(c == COLS - 1))

            s_sb = persist.tile([4, K], f32)
            nc.vector.tensor_copy(s_sb[:], S[:])
            cc2_in = dram.tile([4, K], f32)
            cc2_out = dram.tile([4, K], f32)
            nc.sync.dma_start(cc2_in[:], s_sb[:])
            nc.gpsimd.collective_compute(
                "AllReduce", Op.add,
                replica_groups=[list(range(N_CORES))],
                ins=[cc2_in[:].opt()], outs=[cc2_out[:].opt()])

            # flatten [4, K] -> [1, 4K] on partition 0
            f = persist.tile([1, 4 * K], f32)
            nc.sync.dma_start(
                f[:], cc2_out[:].rearrange("a b -> (a b)").unsqueeze(0))

            ac = f[:, 0:NTH]
            aup = f[:, K:K + NTH]
            t_au = f[:, K + NTH:K + NTH + 1]
            ic = f[:, 2 * K:2 * K + NTH]
            iup = f[:, 3 * K:3 * K + NTH]
            t_iu = f[:, 3 * K + NTH:3 * K + NTH + 1]

            nneg_iu = persist.tile([1, NTH], f32)        # -n_iu
            nc.vector.tensor_scalar(nneg_iu[:], iup, t_iu, None, Op.subtract)
            num = persist.tile([1, NTH], f32)            # n_ac + n_iu
            nc.vector.tensor_sub(num[:], ac, nneg_iu[:])
            nneg_au = persist.tile([1, NTH], f32)        # -n_au
            nc.vector.tensor_scalar(nneg_au[:], aup, t_au, None, Op.subtract)
            den = persist.tile([1, NTH], f32)
            nc.vector.tensor_sub(den[:], num[:], nneg_au[:])
            nc.vector.tensor_add(den[:], den[:], ic)
            nc.vector.tensor_scalar(den[:], den[:], EPS, None, Op.add)
            rden = persist.tile([1, NTH], f32)
            nc.vector.reciprocal(rden[:], den[:])
            evu = persist.tile([1, NTH], f32)
            nc.vector.tensor_mul(evu[:], num[:], rden[:])

            ssum = persist.tile([1, 1], f32)
            nc.vector.reduce_sum(ssum[:], evu[:], axis=mybir.AxisListType.X)
            edge = persist.tile([1, 1], f32)
            nc.vector.tensor_add(edge[:], evu[:, 0:1], evu[:, NTH - 1:NTH])
            nc.vector.tensor_scalar(edge[:], edge[:], 0.5, None, Op.mult)
            auc = persist.tile([1, 1], f32)
            nc.vector.tensor_sub(auc[:], ssum[:], edge[:])
            nc.vector.tensor_scalar(auc[:], auc[:], 1.0 / (2 * (NTH - 1)) * 2,
                                    None, Op.mult)
            eps_t = persist.tile([1, 1], f32)
            nc.vector.memset(eps_t[:], EPS)
            nll = persist.tile([1, 1], f32)
            nc.scalar.activation(nll[:], auc[:],
                                 mybir.ActivationFunctionType.Ln,
                                 bias=eps_t[:])
            res = persist.tile([1, 1], f32)
            nc.vector.tensor_scalar(res[:], nll[:], -1.0, None, Op.mult)
            nc.sync.dma_start(out_d.ap(), res[:])

    nc.compile()
    return nc


_NC = None


def _get_nc():
    global _NC
    if _NC is None:
        _NC = _build_nc()
    return _NC


_VALID = None


def _valid_mask():
    global _VALID
    if _VALID is None:
        v = np.ones((P, COLS), np.float32)
        v[:, MCOLS] = 0.0
        v[:REM, MCOLS] = 1.0
        _VALID = v
    return _VALID


def _in_maps(output, target):
    output = np.ascontiguousarray(np.asarray(output, dtype=np.float32))
    target = np.asarray(target)
    maps = []
    v = _valid_mask()
    for i in range(N_CORES):
        xs = output[i * NPC:(i + 1) * NPC]
        ts = np.asarray(target[i * NPC:(i + 1) * NPC], dtype=np.int64)
        t99 = (99 - ts).astype(np.int32)
        tgt = np.empty((P, COLS), np.int32)
        tgt[:, :MCOLS] = t99[:P * MCOLS].reshape(P, MCOLS)
        tgt[:, MCOLS] = -1
        tgt[:REM, MCOLS] = t99[P * MCOLS:]
        maps.append({"x": xs, "tgt": tgt, "valid": v})
    return maps


def run(output, target, trace=False):
    from concourse.bass_utils import run_bass_kernel_spmd
    nc = _get_nc()
    res = run_bass_kernel_spmd(nc, _in_maps(output, target),
                               core_ids=list(range(N_CORES)), trace=trace)
    val = np.float32(res.results[0]["out"][0, 0])
    return val, res


def kernel(output, target, num_classes):
    assert int(num_classes) == C
    val, _ = run(output, target)
    return np.array(val, dtype=np.float32)
